# revision 42
# baseline (speedup 1.0000x reference)
"""Trainium2 Bass kernel for an 8-expert top-2 MoE layer (expert-parallel, 8 cores).

Per-core algorithm (SPMD; each core owns one expert's weights):
  Phase A (gating, fp32): stream x in 512-token blocks, transpose on PE,
    compute logits^T = Wg^T x^T, top-2 via vector.max, combine weight
    c = w1*[e==arg1] + w2*[e==arg2] (w1 = sigmoid(l1-l2)), and routed mask b.
    A running prefix-sum (triangular matmul + column cumsum) assigns each
    routed token a compact slot pos[t]; x rows are scattered (indirect DMA,
    OOB-skip for non-routed) into a compacted buffer xg[CAP, D].
  Phase B (FFN, f32r/bf16): dense GLU FFN over the CAP compacted rows:
    h = silu(W1^T xg) * (W3^T xg)  (f32r), yg = h^T W2 (bf16), to DRAM.
  Phase C (combine): per 128-token tile, gather yg rows at pos[t]
    (indirect DMA) and scale by c (zero for non-routed) -> y.
Host sums the 8 per-core partial outputs (each token hits exactly 2 experts).
"""
import numpy as np
import ml_dtypes

import concourse.bass as bass
import concourse.bacc as bacc
import concourse.mybir as mybir
import concourse.tile as tile
from concourse.bass_utils import run_bass_kernel_spmd
from concourse.masks import make_identity, make_upper_triangular

dt = mybir.dt
AF = mybir.ActivationFunctionType
ALU = mybir.AluOpType
IOA = bass.IndirectOffsetOnAxis

P = 128
E = 8
BIGPOS = float(1 << 20)


def build(T=8192, D=1024, F=4096, CAP=2304, n_cores=8, sim_safe=False,
          phases="ABC", astop=99):
    NDC = D // P                      # d-chunks
    NFC = F // P                      # f-chunks
    NBLK = T // 512                   # gating blocks of 512 tokens
    NCOL = T // P                     # token columns (tiles of 128)
    ND2 = D // 512                    # output d halves
    fblks = []
    rem = CAP
    while rem > 0:
        b = min(512, rem)
        assert b % 128 == 0
        fblks.append(b)
        rem -= b

    nc = bacc.Bacc("TRN2", target_bir_lowering=False, debug=False)
    x = nc.dram_tensor("x", [T, D], dt.float32, kind="ExternalInput")
    wg = nc.dram_tensor("wg", [P, NDC * E], dt.float32, kind="ExternalInput")
    esel = nc.dram_tensor("esel", [P, E], dt.float32, kind="ExternalInput")
    w1 = nc.dram_tensor("w1", [NFC, P, NDC * P], dt.float32r, kind="ExternalInput")
    w3 = nc.dram_tensor("w3", [NFC, P, NDC * P], dt.float32r, kind="ExternalInput")
    w2 = nc.dram_tensor("w2", [ND2, NFC // 4, P, 4 * 512], dt.bfloat16,
                        kind="ExternalInput")
    y = nc.dram_tensor("y", [T, D], dt.float32, kind="ExternalOutput")
    xg = nc.dram_tensor("xg", [CAP, D], dt.float32, kind="Internal")
    ygs = nc.dram_tensor("ygs", [CAP, D], dt.float32, kind="Internal")

    with tile.TileContext(nc, num_cores=n_cores) as tc:
        with tc.tile_pool(name="res", bufs=1) as res:
            ident = res.tile([P, P], dt.float32)
            make_identity(nc, ident[:])
            tri = res.tile([P, P], dt.float32)        # tri[q,p]=1 iff q<p
            make_upper_triangular(nc, tri[:], val=1.0, diag=False)
            ones_p1 = res.tile([P, 1], dt.float32)
            nc.vector.memset(ones_p1[:], 1.0)
            ones_1p = res.tile([1, P], dt.float32)
            nc.vector.memset(ones_1p[:], 1.0)
            esel_sb = res.tile([P, E], dt.float32)
            nc.sync.dma_start(esel_sb[:], esel[:, :])
            wg_sb = res.tile([P, NDC * E], dt.float32)
            nc.sync.dma_start(wg_sb[:], wg[:, :])
            zeros_x = res.tile([P, D], dt.float32)
            nc.vector.memset(zeros_x[:], 0.0)
            c_all = res.tile([P, NCOL], dt.float32)
            pos_all = res.tile([P, NCOL], dt.int32)
            base_sb = res.tile([1, 1], dt.float32)
            nc.vector.memset(base_sb[:], 0.0)

            # zero-init the compacted buffer (pad rows must be finite)
            for i in range(CAP // P):
                nc.sync.dma_start(xg[i * P:(i + 1) * P, :], zeros_x[:])

            # ---------------- Phase A: gating + dispatch ----------------
            with tc.tile_pool(name="xnA", bufs=6) as xnA, \
                 tc.tile_pool(name="xTA", bufs=9) as xTA, \
                 tc.tile_pool(name="sbA", bufs=2) as sbA, \
                 tc.tile_pool(name="smA", bufs=4) as smA, \
                 tc.tile_pool(name="psA", bufs=2, space="PSUM") as psA, \
                 tc.tile_pool(name="psA2", bufs=1, space="PSUM") as psA2:
                for k in range(NBLK if "A" in phases else 0):
                    xn = []
                    for s in range(4):
                        t = xnA.tile([P, D], dt.float32, tag="xn")
                        nc.sync.dma_start(
                            t[:], x[k * 512 + s * P:k * 512 + (s + 1) * P, :])
                        xn.append(t)
                    xT = []
                    for d in range(NDC):
                        xTd = xTA.tile([P, 512], dt.float32, tag="xT")
                        for s in range(4):
                            tp = psA.tile([P, P], dt.float32, space="PSUM",
                                          tag="tpA")
                            nc.tensor.transpose(
                                tp[:], xn[s][:, d * P:(d + 1) * P], ident[:])
                            nc.vector.tensor_copy(
                                xTd[:, s * P:(s + 1) * P], tp[:])
                        xT.append(xTd)
                    if astop < 2:
                        continue
                    # logits^T [E, 512] (full-partition PSUM tile; write rows 0:E)
                    lgT_ps = psA.tile([P, 512], dt.float32, space="PSUM",
                                      tag="lgT")
                    for d in range(NDC):
                        nc.tensor.matmul(
                            lgT_ps[:E, :], wg_sb[:, d * E:(d + 1) * E],
                            xT[d][:], start=(d == 0), stop=(d == NDC - 1))
                    lgT_sb = sbA.tile([P, 512], dt.float32, tag="lgTs")
                    nc.vector.memset(lgT_sb[:], 0.0)
                    nc.vector.tensor_copy(lgT_sb[:E, :], lgT_ps[:E, :])

                    if astop < 3:
                        continue
                    if astop < 29:
                        continue
                    b_blk = smA.tile([P, 4], dt.float32, tag="bblk")
                    for s in range(4):
                        col = 4 * k + s
                        gtr = psA.tile([P, P], dt.float32, space="PSUM",
                                       tag="gtr")
                        nc.tensor.transpose(
                            gtr[:], lgT_sb[:, s * P:(s + 1) * P], ident[:])
                        if astop < 30:
                            continue
                        lgt = smA.tile([P, E], dt.float32, tag="lgt")
                        nc.vector.tensor_copy(lgt[:], gtr[:, 0:E])
                        lg = lgt[:, 0:E]
                        if astop < 31:
                            continue
                        mx = smA.tile([P, 8], dt.float32, tag="mx")
                        nc.vector.max(mx[:], lg)
                        if astop < 32:
                            continue
                        junk = smA.tile([P, E], dt.float32, tag="junk")
                        le = smA.tile([P, 1], dt.float32, tag="le")
                        nc.vector.tensor_tensor(junk[:], lg, esel_sb[:],
                                                ALU.mult)
                        nc.vector.tensor_reduce(
                            le[:], junk[:], mybir.AxisListType.X, ALU.add)
                        if astop < 33:
                            continue
                        d12 = smA.tile([P, 1], dt.float32, tag="d12")
                        nc.vector.tensor_tensor(
                            d12[:], mx[:, 0:1], mx[:, 1:2], ALU.subtract)
                        w1s = smA.tile([P, 1], dt.float32, tag="w1s")
                        nc.scalar.activation(w1s[:], d12[:], AF.Sigmoid)
                        if astop < 34:
                            continue
                        eq1 = smA.tile([P, 1], dt.float32, tag="eq1")
                        nc.vector.tensor_tensor(
                            eq1[:], le[:], mx[:, 0:1], ALU.is_equal)
                        eq2 = smA.tile([P, 1], dt.float32, tag="eq2")
                        nc.vector.tensor_tensor(
                            eq2[:], le[:], mx[:, 1:2], ALU.is_equal)
                        nc.vector.tensor_tensor(
                            b_blk[:, s:s + 1], eq1[:], eq2[:], ALU.add)
                        tt0 = smA.tile([P, 1], dt.float32, tag="tt0")
                        nc.vector.tensor_tensor(
                            tt0[:], eq1[:], eq2[:], ALU.subtract)
                        tt1 = smA.tile([P, 1], dt.float32, tag="tt1")
                        nc.vector.tensor_tensor(
                            tt1[:], tt0[:], w1s[:], ALU.mult)
                        nc.vector.tensor_tensor(
                            c_all[:, col:col + 1], tt1[:], eq2[:], ALU.add)

                    if astop < 40:
                        continue
                    # block prefix sums -> global positions
                    pblk = psA2.tile([P, 4], dt.float32, space="PSUM",
                                     tag="pblk")
                    nc.tensor.matmul(pblk[:], tri[:], b_blk[:],
                                     start=True, stop=False)
                    tot_ps = psA2.tile([1, 4], dt.float32, space="PSUM",
                                       tag="tot")
                    nc.tensor.matmul(tot_ps[:], ones_p1[:], b_blk[:],
                                     start=True, stop=True)
                    tot = smA.tile([1, 4], dt.float32, tag="tot_sb")
                    nc.vector.tensor_copy(tot[:], tot_ps[:])
                    s1 = smA.tile([1, 4], dt.float32, tag="s1")
                    nc.vector.tensor_copy(s1[:], tot[:])
                    nc.vector.tensor_tensor(
                        s1[:, 1:4], tot[:, 1:4], tot[:, 0:3], ALU.add)
                    s2 = smA.tile([1, 4], dt.float32, tag="s2")
                    nc.vector.tensor_copy(s2[:], s1[:])
                    nc.vector.tensor_tensor(
                        s2[:, 2:4], s1[:, 2:4], s1[:, 0:2], ALU.add)
                    ex = smA.tile([1, 4], dt.float32, tag="ex")
                    nc.vector.memset(ex[:], 0.0)
                    nc.vector.tensor_copy(ex[:, 1:4], s2[:, 0:3])
                    tb = smA.tile([1, 4], dt.float32, tag="tb")
                    nc.vector.tensor_scalar_add(tb[:], ex[:], base_sb[:, 0:1])
                    nc.vector.tensor_tensor(
                        base_sb[:], base_sb[:], s2[:, 3:4], ALU.add)
                    nc.tensor.matmul(pblk[:], ones_1p[:], tb[:],
                                     start=False, stop=True)
                    posf = smA.tile([P, 4], dt.float32, tag="posf")
                    nc.vector.tensor_copy(posf[:], pblk[:])
                    nc.vector.tensor_copy(
                        pos_all[:, 4 * k:4 * k + 4], posf[:])
                    # scatter dests: pos for routed, BIG for others
                    t2 = smA.tile([P, 4], dt.float32, tag="t2")
                    nc.vector.tensor_scalar(
                        t2[:], b_blk[:], -BIGPOS, BIGPOS, ALU.mult, ALU.add)
                    t3 = smA.tile([P, 4], dt.float32, tag="t3")
                    nc.vector.tensor_tensor(t3[:], posf[:], b_blk[:], ALU.mult)
                    t4 = smA.tile([P, 4], dt.float32, tag="t4")
                    nc.vector.tensor_tensor(t4[:], t3[:], t2[:], ALU.add)
                    sci = smA.tile([P, 4], dt.int32, tag="sci")
                    nc.vector.tensor_copy(sci[:], t4[:])
                    if astop < 50:
                        continue
                    for s in range(4):
                        nc.gpsimd.indirect_dma_start(
                            out=xg[:, :],
                            out_offset=IOA(ap=sci[:, s:s + 1], axis=0),
                            in_=xn[s][:], in_offset=None,
                            bounds_check=CAP - 1, oob_is_err=False)

            # ---------------- Phase B: FFN over compacted rows ----------------
            if "B" not in phases:
                fblks_run = []
            else:
                fblks_run = fblks
            with tc.tile_pool(name="xgB", bufs=5) as xgB, \
                 tc.tile_pool(name="xTB", bufs=NDC + 1) as xTB, \
                 tc.tile_pool(name="hTB", bufs=NFC + 2) as hTB, \
                 tc.tile_pool(name="w2B", bufs=max(NFC // 4, 2)) as w2B, \
                 tc.tile_pool(name="sbB", bufs=2) as sbB, \
                 tc.tile_pool(name="psB", bufs=2, space="PSUM") as psB:
                r0 = 0
                for nb in fblks_run:
                    ns = nb // P
                    xgT = []
                    xgn = []
                    for s in range(ns):
                        t = xgB.tile([P, D], dt.float32, tag="xgn")
                        nc.sync.dma_start(
                            t[:], xg[r0 + s * P:r0 + (s + 1) * P, :])
                        xgn.append(t)
                    for d in range(NDC):
                        xTd = xTB.tile([P, nb], dt.float32r, tag="xgT")
                        for s in range(ns):
                            tp = psB.tile([P, P], dt.float32, space="PSUM",
                                          tag="tpB")
                            nc.tensor.transpose(
                                tp[:], xgn[s][:, d * P:(d + 1) * P], ident[:])
                            nc.vector.tensor_copy(
                                xTd[:, s * P:(s + 1) * P], tp[:])
                        xgT.append(xTd)
                    hT = []
                    for f in range(NFC):
                        w1f = sbB.tile([P, D], dt.float32r, tag="w1f")
                        nc.sync.dma_start(w1f[:], w1[f, :, :])
                        w3f = sbB.tile([P, D], dt.float32r, tag="w3f")
                        nc.sync.dma_start(w3f[:], w3[f, :, :])
                        u_ps = psB.tile([P, nb], dt.float32, space="PSUM",
                                        tag="u")
                        for d in range(NDC):
                            nc.tensor.matmul(
                                u_ps[:], w1f[:, d * P:(d + 1) * P], xgT[d][:],
                                start=(d == 0), stop=(d == NDC - 1))
                        v_ps = psB.tile([P, nb], dt.float32, space="PSUM",
                                        tag="v")
                        for d in range(NDC):
                            nc.tensor.matmul(
                                v_ps[:], w3f[:, d * P:(d + 1) * P], xgT[d][:],
                                start=(d == 0), stop=(d == NDC - 1))
                        ssb = sbB.tile([P, nb], dt.float32, tag="ssb")
                        hf = hTB.tile([P, nb], dt.bfloat16, tag="hT")
                        if sim_safe:
                            # CoreSim lacks Silu: silu(u) = u * sigmoid(u)
                            nc.scalar.activation(ssb[:], u_ps[:], AF.Sigmoid)
                            nc.vector.tensor_tensor(ssb[:], ssb[:], u_ps[:],
                                                    ALU.mult)
                        else:
                            nc.scalar.activation(ssb[:], u_ps[:], AF.Silu)
                        nc.vector.tensor_tensor(hf[:], ssb[:], v_ps[:],
                                                ALU.mult)
                        hT.append(hf)
                    for dh in range(ND2):
                        w2g = []
                        for fg in range(NFC // 4):
                            t = w2B.tile([P, 4 * 512], dt.bfloat16, tag="w2g")
                            nc.sync.dma_start(t[:], w2[dh, fg, :, :])
                            w2g.append(t)
                        for s in range(ns):
                            y_ps = psB.tile([P, 512], dt.float32, space="PSUM",
                                            tag="y")
                            for f in range(NFC):
                                nc.tensor.matmul(
                                    y_ps[:],
                                    hT[f][:, s * P:(s + 1) * P],
                                    w2g[f // 4][:, (f % 4) * 512:(f % 4 + 1) * 512],
                                    start=(f == 0), stop=(f == NFC - 1))
                            ysb = sbB.tile([P, 512], dt.float32, tag="ysb")
                            nc.vector.tensor_copy(ysb[:], y_ps[:])
                            nc.sync.dma_start(
                                ygs[r0 + s * P:r0 + (s + 1) * P,
                                    dh * 512:(dh + 1) * 512], ysb[:])
                    r0 += nb

            # ---------------- Phase C: combine ----------------
            with tc.tile_pool(name="sbC", bufs=3) as sbC:
                for c in range(NCOL if "C" in phases else 0):
                    yt = sbC.tile([P, D], dt.float32, tag="yt")
                    nc.gpsimd.indirect_dma_start(
                        out=yt[:], out_offset=None,
                        in_=ygs[:, :],
                        in_offset=IOA(ap=pos_all[:, c:c + 1], axis=0))
                    yo = sbC.tile([P, D], dt.float32, tag="yo")
                    nc.vector.tensor_scalar_mul(yo[:], yt[:],
                                                c_all[:, c:c + 1])
                    nc.sync.dma_start(y[c * P:(c + 1) * P, :], yo[:])

    nc.compile()
    return nc


_module_cache = {}


def _get_module(key, **kw):
    if key not in _module_cache:
        _module_cache[key] = build(**kw)
    return _module_cache[key]


def prep_inputs(x2d, Wg, W1, W3, W2, n_cores=8):
    """Host-side sharding: per-core input maps (core e owns expert e)."""
    T, D = x2d.shape
    E_, Dw, F = W1.shape[0], W1.shape[1], W1.shape[2]
    NDC = D // P
    NFC = F // P
    ND2 = D // 512
    wg_l = np.ascontiguousarray(
        Wg.reshape(NDC, P, E).transpose(1, 0, 2).reshape(P, NDC * E)
        .astype(np.float32))
    in_maps = []
    for e in range(n_cores):
        w1e = np.ascontiguousarray(
            W1[e].reshape(NDC, P, NFC, P).transpose(2, 1, 0, 3)
            .reshape(NFC, P, NDC * P))
        w3e = np.ascontiguousarray(
            W3[e].reshape(NDC, P, NFC, P).transpose(2, 1, 0, 3)
            .reshape(NFC, P, NDC * P))
        w2e = np.ascontiguousarray(
            W2[e].reshape(NFC, P, ND2, 512).transpose(2, 0, 1, 3)
            .reshape(ND2, NFC // 4, 4, P, 512).transpose(0, 1, 3, 2, 4)
            .reshape(ND2, NFC // 4, P, 4 * 512)
            .astype(ml_dtypes.bfloat16))
        ese = np.zeros((P, E), np.float32)
        ese[:, e] = 1.0
        in_maps.append({
            "x": x2d, "wg": wg_l, "esel": ese,
            "w1": w1e, "w3": w3e, "w2": w2e,
        })
    return in_maps


def kernel(x, Wg, W1, W2, W3):
    x = np.asarray(x, np.float32)
    Wg = np.asarray(Wg, np.float32)
    W1 = np.asarray(W1, np.float32)
    W2 = np.asarray(W2, np.float32)
    W3 = np.asarray(W3, np.float32)
    Bb, Ll, D = x.shape
    T = Bb * Ll
    F = W1.shape[2]
    x2d = np.ascontiguousarray(x.reshape(T, D))
    nc = _get_module("full", T=T, D=D, F=F, CAP=2304, n_cores=8)
    in_maps = prep_inputs(x2d, Wg, W1, W3, W2, n_cores=8)
    res = run_bass_kernel_spmd(nc, in_maps, core_ids=list(range(8)))
    out = np.zeros((T, D), np.float64)
    for r in res.results:
        out += r["y"].astype(np.float64)
    return out.astype(np.float32).reshape(Bb, Ll, D)
